# revision 19
# baseline (speedup 1.0000x reference)
"""DagEncoder (MLP + segment_sum) Trainium2 kernel, 8-core SPMD.

Contract: kernel(**inputs) takes the FULL unsharded inputs of
reference.setup_inputs() and returns the FULL [M, E] output.

Strategy (pure data parallelism over DAG segments):
  - 20000 segments split into 8 cores x 2500 segments; each core's segments
    are split into 2 "streams" at a node-count midpoint so two nodes are
    processed per PE column (feature-major layout, 2x40 features stacked on
    partitions 0..79).
  - Nodes are grouped into blocks of B=8 (per segment, zero-padded to a
    multiple of 8). Blocks are packed into regions of W columns; node s of
    block b lives at column b of sub-chunk s, so the 8:1 block reduction is
    expressed as 8 successive ops on the SAME [128, W] accumulator:
      s=0: ACT  acc_a = relu(p2 + b2)            (activation, exact bias)
      s=1: DVE  acc_b = relu(p2 + b2)            (tensor_scalar)
      s>=2: DVE acc_{a/b} = max(p2,0) + acc      (scalar_tensor_tensor;
                                                  requires b2 == 0, which
                                                  holds for this problem;
                                                  a 2-instr fallback covers
                                                  the general case)
  - Per 1024-col chunk: mm1 (W1 blockdiag, 80->128) -> ACT relu evac h1
    (exact b1) -> mm2 (W2 blockdiag) -> h2 consume as above. GpSimd combines
    acc_a + acc_b per region; block partial sums are DMA'd out.
  - Host: per-segment sums of block partials (cumsum-diff; blocks of one
    segment are consecutive), then @W3 + counts*b3 (linear ops commute with
    the segment sum), plus a pad-slot correction (zero for zero biases).
"""

import sys
import types

sys.path.insert(0, "/opt/trn_rl_repo")

import numpy as np
import ml_dtypes

import concourse.bass as bass  # noqa: F401  (side-effect imports)
import concourse.bacc as bacc
import concourse.mybir as mybir
import concourse.tile as tile
from concourse.bass_utils import run_bass_kernel_spmd

BF16 = ml_dtypes.bfloat16

NCORES = 8
B = 8            # nodes per block (segment padding unit, = fold depth)
W_FULL = 1024    # blocks per full region (acc width)
W_GRAN = 256     # tail-region width granularity
CHUNK = 1024     # psum chunk columns

# Stash of the last run's BassKernelResults for the dev harness.
LAST_RESULT = None


# ----------------------------------------------------------------------------
# Host-side layout
# ----------------------------------------------------------------------------

def _stream_bounds(cnts):
    """Split all segments into 2*NCORES contiguous ranges with near-equal
    block counts (core c gets streams 2c and 2c+1)."""
    nb = -(-cnts // B)
    tot = int(nb.sum())
    cum = np.concatenate([[0], np.cumsum(nb)])
    ns = 2 * NCORES
    bounds = [int(np.searchsorted(cum, round(tot * j / ns)))
              for j in range(ns + 1)]
    bounds[0], bounds[-1] = 0, len(cnts)
    return bounds


def _stream_blocks(starts, cnts):
    """Block arrays for one stream: (blk_src, blk_cnt, nb_per_seg)."""
    nb_per_seg = -(-cnts // B)          # ceil; 0 for empty segments
    nblocks = int(nb_per_seg.sum())
    seg_of_blk = np.repeat(np.arange(len(cnts)), nb_per_seg)
    blk_starts = np.concatenate([[0], np.cumsum(nb_per_seg)])
    within = np.arange(nblocks) - blk_starts[seg_of_blk]
    blk_src = np.repeat(starts, nb_per_seg) + B * within
    blk_cnt = np.minimum(B, np.repeat(cnts, nb_per_seg) - B * within)
    return blk_src, blk_cnt, nb_per_seg


def _region_plan(blocks_padded):
    """List of (blk_off, W) regions covering blocks_padded block columns.
    A small runt region leads (fast pipeline ramp: its first chunk's DMA is
    tiny) and one trails (short drain: the final accumulate chain is short).
    """
    plan = []
    off = 0
    runts = []
    if blocks_padded >= 4 * W_FULL:
        runts = [W_GRAN, W_FULL - W_GRAN]          # leading 256 + 768
        tail_runts = [W_FULL - W_GRAN, 192, 64]    # short final drain chain
    else:
        tail_runts = []
    for w in runts:
        plan.append((off, w))
        off += w
    mid_end = blocks_padded - sum(tail_runts)
    while off < mid_end:
        w = min(W_FULL, mid_end - off)
        plan.append((off, w))
        off += w
    for w in tail_runts:
        plan.append((off, w))
        off += w
    return plan


def _col_src(blk_src, blk_cnt, plan):
    """node source index per xcat column (-1 = zero pad) for the layout:
    region r, sub-chunk s (0..B-1), col c -> node s of block blk_off+c."""
    total_blocks = sum(w for _, w in plan)
    out = np.empty(total_blocks * B, np.int64)
    col = 0
    for blk_off, w in plan:
        bs = blk_src[blk_off:blk_off + w]
        bc = blk_cnt[blk_off:blk_off + w]
        s = np.arange(B)[:, None]
        srcs = bs[None, :] + s
        valid = (bs[None, :] >= 0) & (s < bc[None, :])
        out[col:col + B * w] = np.where(valid, srcs, -1).reshape(-1)
        col += B * w
    return out


def _gather_T(a, src):
    """a[src].T with src == -1 rows zeroed; returns [a.shape[1], len(src)]
    as bf16."""
    g = a[np.clip(src, 0, a.shape[0] - 1)]
    g[src < 0] = 0
    return np.ascontiguousarray(g.T.astype(BF16))


# ----------------------------------------------------------------------------
# Device program
# ----------------------------------------------------------------------------

def _build_device_program(plan, zero_b2):
    """Trace the Bass program for the given region plan."""
    dt = mybir.dt
    AL = mybir.AluOpType
    ACTF = mybir.ActivationFunctionType

    TOTB = sum(w for _, w in plan)   # total block columns
    C = TOTB * B                     # xcat columns

    nc = bacc.Bacc(None, target_bir_lowering=False)

    xcat = nc.dram_tensor("xcat", [80, C], dt.bfloat16, kind="ExternalInput")
    w1 = nc.dram_tensor("w1blk", [80, 128], dt.bfloat16, kind="ExternalInput")
    w2 = nc.dram_tensor("w2blk", [128, 128], dt.bfloat16, kind="ExternalInput")
    b1s = nc.dram_tensor("b1s", [128, 1], dt.float32, kind="ExternalInput")
    b2s = nc.dram_tensor("b2s", [128, 1], dt.float32, kind="ExternalInput")
    outT = nc.dram_tensor("outT", [128, TOTB], dt.bfloat16,
                          kind="ExternalOutput")
    w_last = plan[-1][1]
    outA = nc.dram_tensor("outA", [128, w_last], dt.bfloat16,
                          kind="ExternalOutput")
    outB = nc.dram_tensor("outB", [128, w_last], dt.bfloat16,
                          kind="ExternalOutput")

    from contextlib import ExitStack

    with tile.TileContext(nc) as tc, ExitStack() as ctx:
        consts = ctx.enter_context(tc.tile_pool(name="consts", bufs=1))
        xin_pool = ctx.enter_context(tc.tile_pool(name="xin", bufs=4))
        h1_pool = ctx.enter_context(tc.tile_pool(name="h1p", bufs=4))
        acc_pool = ctx.enter_context(tc.tile_pool(name="accp", bufs=3))
        out_pool = ctx.enter_context(tc.tile_pool(name="outp", bufs=3))
        psum = ctx.enter_context(tc.tile_pool(name="psum", bufs=2, space="PSUM"))

        NR = len(plan)

        # chunk list: (region, s, col_off, width)
        chunks = []
        col = 0
        for r, (blk_off, w) in enumerate(plan):
            for s in range(B):
                chunks.append((r, s, col, w))
                col += w
        NQ = len(chunks)

        xts = {}       # region -> xcat SBUF tile
        accs = {}      # region -> (acc_a, acc_b)
        p1s = {}       # q -> p1 psum tile (pending mm1 -> h1 evac)
        p2s = {}       # q -> p2 psum tile (pending h2 consume)
        h1s = {}       # q -> h1 SBUF tile

        def dma_in(r):
            blk_off, w = plan[r]
            cw = B * w
            xt = xin_pool.tile([80, cw], dt.bfloat16, tag="xt",
                               name=f"xt_{r}")
            # split across DMA rings for parallelism + faster first chunk
            if cw % 4096 == 0:
                npc = 4
            elif cw % 3072 == 0:
                npc = 3
            elif cw % 2048 == 0:
                npc = 2
            else:
                npc = 1
            pw = cw // npc
            for p in range(npc):
                nc.sync.dma_start(
                    xt[:, p * pw:(p + 1) * pw],
                    xcat[:, blk_off * B + p * pw:blk_off * B + (p + 1) * pw])
            xts[r] = xt

        def emit_mm1(q):
            r, s, co, w = chunks[q]
            p1 = psum.tile([128, w], dt.float32, tag="p1", name=f"p1_{q}")
            xt = xts[r]
            xo = (co - plan[r][0] * B)
            for o in range(0, w, 512):
                n = min(512, w - o)
                nc.tensor.matmul(p1[:, o:o + n], w1t[:],
                                 xt[:, xo + o:xo + o + n],
                                 start=True, stop=True)
            p1s[q] = p1

        # fine ACT/DVE balance: split one h1 evac per 3 full regions
        split_h1 = set()
        full_idx = 0
        for qq, (r, s, co, w) in enumerate(chunks):
            if w == W_FULL and s == 4:
                if False:  # disabled: measured +9% on DVE op durations
                    split_h1.add(qq)
                full_idx += 1

        def emit_h1_mm2(q):
            r, s, co, w = chunks[q]
            p1 = p1s.pop(q)
            h1 = h1_pool.tile([128, w], dt.bfloat16, tag="h1", name=f"h1_{q}")
            if q in split_h1:
                nc.scalar.activation(h1[:, :512], p1[:, :512], ACTF.Relu,
                                     bias=b1t[:], scale=1.0)
                nc.vector.tensor_scalar(h1[:, 512:], p1[:, 512:], b1t[:],
                                        0.0, AL.add, AL.max)
            else:
                nc.scalar.activation(h1[:], p1[:], ACTF.Relu, bias=b1t[:],
                                     scale=1.0)
            p2 = psum.tile([128, w], dt.float32, tag="p2", name=f"p2_{q}")
            for o in range(0, w, 512):
                n = min(512, w - o)
                nc.tensor.matmul(p2[:, o:o + n], w2t[:], h1[:, o:o + n],
                                 start=True, stop=True)
            p2s[q] = p2
            h1s[q] = h1

        def emit_h2(q):
            r, s, co, w = chunks[q]
            p2 = p2s.pop(q)
            h1s.pop(q, None)
            last_r = r == NR - 1
            if s == 0:
                acc_a = acc_pool.tile([128, w], dt.bfloat16, tag="acca",
                                      name=f"acca_{r}")
                acc_b = acc_pool.tile([128, w], dt.bfloat16, tag="accb",
                                      name=f"accb_{r}")
                accs[r] = (acc_a, acc_b)
                if w > 512 and r % 2 == 0:
                    # split at the psum bank boundary: ACT bank 0, DVE bank 1
                    nc.scalar.activation(acc_a[:, :512], p2[:, :512],
                                         ACTF.Relu, bias=b2t[:], scale=1.0)
                    nc.vector.tensor_scalar(acc_a[:, 512:], p2[:, 512:],
                                            b2t[:], 0.0, AL.add, AL.max)
                elif r % 2 == 1:
                    # odd regions: s0 entirely on DVE (ACT is the hotter
                    # engine; this alternation balances their totals)
                    nc.vector.tensor_scalar(acc_a[:], p2[:], b2t[:], 0.0,
                                            AL.add, AL.max)
                else:
                    nc.scalar.activation(acc_a[:], p2[:], ACTF.Relu,
                                         bias=b2t[:], scale=1.0)
                return
            acc_a, acc_b = accs[r]
            acc = acc_a if s % 2 == 0 else acc_b
            if s == 1:
                nc.vector.tensor_scalar(acc[:], p2[:], b2t[:], 0.0,
                                        AL.add, AL.max)
            elif zero_b2:
                nc.vector.scalar_tensor_tensor(acc[:], p2[:], 0.0, acc[:],
                                               AL.max, AL.add)
            else:
                tmp = h1_pool.tile([128, w], dt.bfloat16, tag="tmp",
                                   name=f"tmp_{q}")
                nc.vector.tensor_scalar(tmp[:], p2[:], b2t[:], 0.0,
                                        AL.add, AL.max)
                nc.vector.tensor_tensor(acc[:], tmp[:], acc[:], op=AL.add)
            if s == B - 1:
                blk_off, _ = plan[r]
                if last_r:
                    # drain fast: ship both accumulators, host adds them
                    nc.sync.dma_start(outA[:, :w], acc_a[:])
                    nc.sync.dma_start(outB[:, :w], acc_b[:])
                else:
                    oc = out_pool.tile([128, w], dt.bfloat16, tag="oc",
                                       name=f"oc_{r}")
                    nc.gpsimd.tensor_tensor(oc[:], acc_a[:], acc_b[:],
                                            op=AL.add)
                    nc.sync.dma_start(outT[:, blk_off:blk_off + w], oc[:])
                accs.pop(r)

        # prologue: region-0 data first (it gates the first matmul), then
        # weights/biases, then deeper prefetch
        dma_in(0)
        w1t = consts.tile([80, 128], dt.bfloat16)
        nc.sync.dma_start(w1t[:], w1[:])
        w2t = consts.tile([128, 128], dt.bfloat16)
        nc.sync.dma_start(w2t[:], w2[:])
        b1t = consts.tile([128, 1], dt.float32)
        nc.sync.dma_start(b1t[:], b1s[:])
        b2t = consts.tile([128, 1], dt.float32)
        nc.sync.dma_start(b2t[:], b2s[:])
        for r in range(1, min(3, NR)):
            dma_in(r)
        emit_mm1(0)
        # steady state: iteration q emits mm1(q+1), h1+mm2(q), h2(q-1)
        for q in range(NQ):
            r, s, co, w = chunks[q]
            # prefetch region r+3 once every mm1 read of region r is emitted
            # (its xin buffer slot is the one being recycled)
            if s == B - 1 and r + 3 < NR:
                dma_in(r + 3)
            if q + 1 < NQ:
                emit_mm1(q + 1)
            emit_h1_mm2(q)
            if q > 0:
                emit_h2(q - 1)
        emit_h2(NQ - 1)

    nc.finalize()
    return nc


# ----------------------------------------------------------------------------
# Entry point
# ----------------------------------------------------------------------------

def _maybe_install_ntff_hook():
    try:
        import antenv.axon_hooks  # noqa: F401
        return
    except ImportError:
        pass
    try:
        from trn_agent_boot.trn_boot import _ntff_profile_via_ctypes
        hook = _ntff_profile_via_ctypes("/opt/axon/libaxon_pjrt.so")
        mod = types.ModuleType("antenv.axon_hooks")
        mod.get_axon_ntff_profile_hook = lambda: hook
        mod.set_axon_ntff_profile_hook = lambda h: None
        sys.modules["antenv.axon_hooks"] = mod
    except Exception:
        pass


def kernel(x, h_node, W1, b1, W2, b2, W3, b3, ptr):
    global LAST_RESULT
    x = np.asarray(x, np.float32)
    h_node = np.asarray(h_node, np.float32)
    W1 = np.asarray(W1, np.float32)
    W2 = np.asarray(W2, np.float32)
    W3 = np.asarray(W3, np.float32)
    b1 = np.asarray(b1, np.float32)
    b2 = np.asarray(b2, np.float32)
    b3 = np.asarray(b3, np.float32)
    ptr = np.asarray(ptr).astype(np.int64)
    N, F = x.shape
    E = h_node.shape[1]
    M = ptr.shape[0] - 1

    cnts = np.diff(ptr)

    # per-core/stream block arrays and the common padded block count
    bounds = _stream_bounds(cnts)
    core_streams = []
    blk_max = 0
    for c in range(NCORES):
        streams = []
        for st in range(2):
            l2, h2 = bounds[2 * c + st], bounds[2 * c + st + 1]
            blk_src, blk_cnt, nb_per_seg = _stream_blocks(
                ptr[l2:h2], cnts[l2:h2])
            streams.append((l2, h2, blk_src, blk_cnt, nb_per_seg))
            blk_max = max(blk_max, len(blk_src))
        core_streams.append(streams)

    blocks_padded = -(-blk_max // W_GRAN) * W_GRAN
    plan = _region_plan(blocks_padded)
    TOTB = blocks_padded
    C = TOTB * B

    # device weight/constant tensors
    w1blk = np.zeros((80, 128), np.float32)
    w1blk[0:40, 0:64] = W1
    w1blk[40:80, 64:128] = W1
    w2blk = np.zeros((128, 128), np.float32)
    w2blk[0:64, 0:64] = W2
    w2blk[64:128, 64:128] = W2
    b1st = np.concatenate([b1, b1]).reshape(128, 1).astype(np.float32)
    b2st = np.concatenate([b2, b2]).reshape(128, 1).astype(np.float32)

    in_maps = []
    for c in range(NCORES):
        xcat = np.zeros((80, C), BF16)
        for st, (l2, h2, blk_src, blk_cnt, nb_per_seg) in \
                enumerate(core_streams[c]):
            bs = np.full(TOTB, -1, np.int64)
            bc = np.zeros(TOTB, np.int64)
            bs[:len(blk_src)] = blk_src
            bc[:len(blk_cnt)] = blk_cnt
            src = _col_src(bs, bc, plan)
            r0 = 40 * st
            xcat[r0:r0 + 8, :] = _gather_T(x, src)
            xcat[r0 + 8:r0 + 40, :] = _gather_T(h_node, src)
        in_maps.append({
            "xcat": xcat,
            "w1blk": w1blk.astype(BF16),
            "w2blk": w2blk.astype(BF16),
            "b1s": b1st,
            "b2s": b2st,
        })

    zero_b2 = bool(np.all(b2 == 0.0))
    nc = _build_device_program(plan, zero_b2)
    _maybe_install_ntff_hook()
    res = run_bass_kernel_spmd(nc, in_maps, core_ids=list(range(NCORES)))
    LAST_RESULT = res

    # host assembly: block partials -> segment sums -> @W3 + bias terms
    out = np.zeros((M, E), np.float32)
    # each empty slot inside a real block contributed relu(relu(b1)@W2 + b2)
    h2c = np.maximum(np.maximum(b1, 0.0) @ W2 + b2, 0.0)
    corr = (h2c @ W3).astype(np.float32)  # [E]
    last_off, w_last = plan[-1]
    for c in range(NCORES):
        P = np.asarray(res.results[c]["outT"], np.float32)  # [128, TOTB]
        P[:, last_off:last_off + w_last] = (
            np.asarray(res.results[c]["outA"], np.float32)
            + np.asarray(res.results[c]["outB"], np.float32))
        for st, (l2, h2, blk_src, blk_cnt, nb_per_seg) in \
                enumerate(core_streams[c]):
            nb = len(blk_src)
            p_st = P[st * 64:(st + 1) * 64, :nb].T  # [nb, 64]
            cs = np.concatenate([np.zeros((1, 64), np.float64),
                                 np.cumsum(p_st, axis=0, dtype=np.float64)])
            ends = np.cumsum(nb_per_seg)
            starts = ends - nb_per_seg
            h2sum = (cs[ends] - cs[starts]).astype(np.float32)  # [nsegs, 64]
            segs = np.arange(l2, h2)
            pad_slots = (B * nb_per_seg - cnts[l2:h2]).astype(np.float32)
            out[segs] = (h2sum @ W3
                         + cnts[l2:h2, None].astype(np.float32) * b3[None, :]
                         - pad_slots[:, None] * corr[None, :])
    return out


# revision 20
# speedup vs baseline: 1.0149x; 1.0149x over previous
"""DagEncoder (MLP + segment_sum) Trainium2 kernel, 8-core SPMD.

Contract: kernel(**inputs) takes the FULL unsharded inputs of
reference.setup_inputs() and returns the FULL [M, E] output.

Strategy (pure data parallelism over DAG segments):
  - 20000 segments split into 8 cores x 2500 segments; each core's segments
    are split into 2 "streams" at a node-count midpoint so two nodes are
    processed per PE column (feature-major layout, 2x40 features stacked on
    partitions 0..79).
  - Nodes are grouped into blocks of B=8 (per segment, zero-padded to a
    multiple of 8). Blocks are packed into regions of W columns; node s of
    block b lives at column b of sub-chunk s, so the 8:1 block reduction is
    expressed as 8 successive ops on the SAME [128, W] accumulator:
      s=0: ACT  acc_a = relu(p2 + b2)            (activation, exact bias)
      s=1: DVE  acc_b = relu(p2 + b2)            (tensor_scalar)
      s>=2: DVE acc_{a/b} = max(p2,0) + acc      (scalar_tensor_tensor;
                                                  requires b2 == 0, which
                                                  holds for this problem;
                                                  a 2-instr fallback covers
                                                  the general case)
  - Per 1024-col chunk: mm1 (W1 blockdiag, 80->128) -> ACT relu evac h1
    (exact b1) -> mm2 (W2 blockdiag) -> h2 consume as above. GpSimd combines
    acc_a + acc_b per region; block partial sums are DMA'd out.
  - Host: per-segment sums of block partials (cumsum-diff; blocks of one
    segment are consecutive), then @W3 + counts*b3 (linear ops commute with
    the segment sum), plus a pad-slot correction (zero for zero biases).
"""

import sys
import types

sys.path.insert(0, "/opt/trn_rl_repo")

import numpy as np
import ml_dtypes

import concourse.bass as bass  # noqa: F401  (side-effect imports)
import concourse.bacc as bacc
import concourse.mybir as mybir
import concourse.tile as tile
from concourse.bass_utils import run_bass_kernel_spmd

BF16 = ml_dtypes.bfloat16

NCORES = 8
B = 8            # nodes per block (segment padding unit, = fold depth)
W_FULL = 1024    # blocks per full region (acc width)
W_GRAN = 256     # tail-region width granularity
CHUNK = 1024     # psum chunk columns

# Stash of the last run's BassKernelResults for the dev harness.
LAST_RESULT = None


# ----------------------------------------------------------------------------
# Host-side layout
# ----------------------------------------------------------------------------

def _stream_bounds(cnts):
    """Split all segments into 2*NCORES contiguous ranges with near-equal
    block counts (core c gets streams 2c and 2c+1)."""
    nb = -(-cnts // B)
    tot = int(nb.sum())
    cum = np.concatenate([[0], np.cumsum(nb)])
    ns = 2 * NCORES
    bounds = [int(np.searchsorted(cum, round(tot * j / ns)))
              for j in range(ns + 1)]
    bounds[0], bounds[-1] = 0, len(cnts)
    return bounds


def _stream_blocks(starts, cnts):
    """Block arrays for one stream: (blk_src, blk_cnt, nb_per_seg)."""
    nb_per_seg = -(-cnts // B)          # ceil; 0 for empty segments
    nblocks = int(nb_per_seg.sum())
    seg_of_blk = np.repeat(np.arange(len(cnts)), nb_per_seg)
    blk_starts = np.concatenate([[0], np.cumsum(nb_per_seg)])
    within = np.arange(nblocks) - blk_starts[seg_of_blk]
    blk_src = np.repeat(starts, nb_per_seg) + B * within
    blk_cnt = np.minimum(B, np.repeat(cnts, nb_per_seg) - B * within)
    return blk_src, blk_cnt, nb_per_seg


def _region_plan(blocks_padded):
    """List of (blk_off, W) regions covering blocks_padded block columns.
    A small runt region leads (fast pipeline ramp: its first chunk's DMA is
    tiny) and one trails (short drain: the final accumulate chain is short).
    """
    plan = []
    off = 0
    runts = []
    if blocks_padded >= 4 * W_FULL:
        runts = [W_GRAN, W_FULL - W_GRAN]          # leading 256 + 768
        tail_runts = [W_FULL - W_GRAN, W_GRAN]     # trailing 768 + 256
    else:
        tail_runts = []
    for w in runts:
        plan.append((off, w))
        off += w
    mid_end = blocks_padded - sum(tail_runts)
    while off < mid_end:
        w = min(W_FULL, mid_end - off)
        plan.append((off, w))
        off += w
    for w in tail_runts:
        plan.append((off, w))
        off += w
    return plan


def _col_src(blk_src, blk_cnt, plan):
    """node source index per xcat column (-1 = zero pad) for the layout:
    region r, sub-chunk s (0..B-1), col c -> node s of block blk_off+c."""
    total_blocks = sum(w for _, w in plan)
    out = np.empty(total_blocks * B, np.int64)
    col = 0
    for blk_off, w in plan:
        bs = blk_src[blk_off:blk_off + w]
        bc = blk_cnt[blk_off:blk_off + w]
        s = np.arange(B)[:, None]
        srcs = bs[None, :] + s
        valid = (bs[None, :] >= 0) & (s < bc[None, :])
        out[col:col + B * w] = np.where(valid, srcs, -1).reshape(-1)
        col += B * w
    return out


def _gather_T(a, src):
    """a[src].T with src == -1 rows zeroed; returns [a.shape[1], len(src)]
    as bf16."""
    g = a[np.clip(src, 0, a.shape[0] - 1)]
    g[src < 0] = 0
    return np.ascontiguousarray(g.T.astype(BF16))


# ----------------------------------------------------------------------------
# Device program
# ----------------------------------------------------------------------------

def _build_device_program(plan, zero_b2):
    """Trace the Bass program for the given region plan."""
    dt = mybir.dt
    AL = mybir.AluOpType
    ACTF = mybir.ActivationFunctionType

    TOTB = sum(w for _, w in plan)   # total block columns
    C = TOTB * B                     # xcat columns

    nc = bacc.Bacc(None, target_bir_lowering=False)

    xcat = nc.dram_tensor("xcat", [80, C], dt.bfloat16, kind="ExternalInput")
    w1 = nc.dram_tensor("w1blk", [80, 128], dt.bfloat16, kind="ExternalInput")
    w2 = nc.dram_tensor("w2blk", [128, 128], dt.bfloat16, kind="ExternalInput")
    b1s = nc.dram_tensor("b1s", [128, 1], dt.float32, kind="ExternalInput")
    b2s = nc.dram_tensor("b2s", [128, 1], dt.float32, kind="ExternalInput")
    outT = nc.dram_tensor("outT", [128, TOTB], dt.bfloat16,
                          kind="ExternalOutput")
    w_last = plan[-1][1]
    outA = nc.dram_tensor("outA", [128, w_last], dt.bfloat16,
                          kind="ExternalOutput")
    outB = nc.dram_tensor("outB", [128, w_last], dt.bfloat16,
                          kind="ExternalOutput")

    from contextlib import ExitStack

    with tile.TileContext(nc) as tc, ExitStack() as ctx:
        consts = ctx.enter_context(tc.tile_pool(name="consts", bufs=1))
        xin_pool = ctx.enter_context(tc.tile_pool(name="xin", bufs=4))
        h1_pool = ctx.enter_context(tc.tile_pool(name="h1p", bufs=4))
        acc_pool = ctx.enter_context(tc.tile_pool(name="accp", bufs=3))
        out_pool = ctx.enter_context(tc.tile_pool(name="outp", bufs=3))
        psum = ctx.enter_context(tc.tile_pool(name="psum", bufs=2, space="PSUM"))

        NR = len(plan)

        # chunk list: (region, s, col_off, width)
        chunks = []
        col = 0
        for r, (blk_off, w) in enumerate(plan):
            for s in range(B):
                chunks.append((r, s, col, w))
                col += w
        NQ = len(chunks)

        xts = {}       # region -> xcat SBUF tile
        accs = {}      # region -> (acc_a, acc_b)
        p1s = {}       # q -> p1 psum tile (pending mm1 -> h1 evac)
        p2s = {}       # q -> p2 psum tile (pending h2 consume)
        h1s = {}       # q -> h1 SBUF tile

        def dma_in(r):
            blk_off, w = plan[r]
            cw = B * w
            xt = xin_pool.tile([80, cw], dt.bfloat16, tag="xt",
                               name=f"xt_{r}")
            # split across DMA rings for parallelism + faster first chunk
            if cw % 4096 == 0:
                npc = 4
            elif cw % 3072 == 0:
                npc = 3
            elif cw % 2048 == 0:
                npc = 2
            else:
                npc = 1
            pw = cw // npc
            for p in range(npc):
                nc.sync.dma_start(
                    xt[:, p * pw:(p + 1) * pw],
                    xcat[:, blk_off * B + p * pw:blk_off * B + (p + 1) * pw])
            xts[r] = xt

        def emit_mm1(q):
            r, s, co, w = chunks[q]
            p1 = psum.tile([128, w], dt.float32, tag="p1", name=f"p1_{q}")
            xt = xts[r]
            xo = (co - plan[r][0] * B)
            for o in range(0, w, 512):
                n = min(512, w - o)
                nc.tensor.matmul(p1[:, o:o + n], w1t[:],
                                 xt[:, xo + o:xo + o + n],
                                 start=True, stop=True)
            p1s[q] = p1

        # fine ACT/DVE balance: split one h1 evac per 3 full regions
        split_h1 = set()
        full_idx = 0
        for qq, (r, s, co, w) in enumerate(chunks):
            if w == W_FULL and s == 4:
                if False:  # disabled: measured +9% on DVE op durations
                    split_h1.add(qq)
                full_idx += 1

        def emit_h1_mm2(q):
            r, s, co, w = chunks[q]
            p1 = p1s.pop(q)
            h1 = h1_pool.tile([128, w], dt.bfloat16, tag="h1", name=f"h1_{q}")
            if q in split_h1:
                nc.scalar.activation(h1[:, :512], p1[:, :512], ACTF.Relu,
                                     bias=b1t[:], scale=1.0)
                nc.vector.tensor_scalar(h1[:, 512:], p1[:, 512:], b1t[:],
                                        0.0, AL.add, AL.max)
            else:
                nc.scalar.activation(h1[:], p1[:], ACTF.Relu, bias=b1t[:],
                                     scale=1.0)
            p2 = psum.tile([128, w], dt.float32, tag="p2", name=f"p2_{q}")
            for o in range(0, w, 512):
                n = min(512, w - o)
                nc.tensor.matmul(p2[:, o:o + n], w2t[:], h1[:, o:o + n],
                                 start=True, stop=True)
            p2s[q] = p2
            h1s[q] = h1

        def emit_h2(q):
            r, s, co, w = chunks[q]
            p2 = p2s.pop(q)
            h1s.pop(q, None)
            last_r = r == NR - 1
            if s == 0:
                acc_a = acc_pool.tile([128, w], dt.bfloat16, tag="acca",
                                      name=f"acca_{r}")
                acc_b = acc_pool.tile([128, w], dt.bfloat16, tag="accb",
                                      name=f"accb_{r}")
                accs[r] = (acc_a, acc_b)
                if w > 512 and r % 2 == 0:
                    # split at the psum bank boundary: ACT bank 0, DVE bank 1
                    nc.scalar.activation(acc_a[:, :512], p2[:, :512],
                                         ACTF.Relu, bias=b2t[:], scale=1.0)
                    nc.vector.tensor_scalar(acc_a[:, 512:], p2[:, 512:],
                                            b2t[:], 0.0, AL.add, AL.max)
                elif r % 2 == 1:
                    # odd regions: s0 entirely on DVE (ACT is the hotter
                    # engine; this alternation balances their totals)
                    nc.vector.tensor_scalar(acc_a[:], p2[:], b2t[:], 0.0,
                                            AL.add, AL.max)
                else:
                    nc.scalar.activation(acc_a[:], p2[:], ACTF.Relu,
                                         bias=b2t[:], scale=1.0)
                return
            acc_a, acc_b = accs[r]
            acc = acc_a if s % 2 == 0 else acc_b
            if s == 1:
                nc.vector.tensor_scalar(acc[:], p2[:], b2t[:], 0.0,
                                        AL.add, AL.max)
            elif zero_b2:
                nc.vector.scalar_tensor_tensor(acc[:], p2[:], 0.0, acc[:],
                                               AL.max, AL.add)
            else:
                tmp = h1_pool.tile([128, w], dt.bfloat16, tag="tmp",
                                   name=f"tmp_{q}")
                nc.vector.tensor_scalar(tmp[:], p2[:], b2t[:], 0.0,
                                        AL.add, AL.max)
                nc.vector.tensor_tensor(acc[:], tmp[:], acc[:], op=AL.add)
            if s == B - 1:
                blk_off, _ = plan[r]
                if last_r:
                    # drain fast: ship both accumulators, host adds them
                    nc.sync.dma_start(outA[:, :w], acc_a[:])
                    nc.sync.dma_start(outB[:, :w], acc_b[:])
                else:
                    oc = out_pool.tile([128, w], dt.bfloat16, tag="oc",
                                       name=f"oc_{r}")
                    nc.gpsimd.tensor_tensor(oc[:], acc_a[:], acc_b[:],
                                            op=AL.add)
                    nc.sync.dma_start(outT[:, blk_off:blk_off + w], oc[:])
                accs.pop(r)

        # prologue: region-0 data first (it gates the first matmul), then
        # weights/biases, then deeper prefetch
        dma_in(0)
        w1t = consts.tile([80, 128], dt.bfloat16)
        nc.sync.dma_start(w1t[:], w1[:])
        w2t = consts.tile([128, 128], dt.bfloat16)
        nc.sync.dma_start(w2t[:], w2[:])
        b1t = consts.tile([128, 1], dt.float32)
        nc.sync.dma_start(b1t[:], b1s[:])
        b2t = consts.tile([128, 1], dt.float32)
        nc.sync.dma_start(b2t[:], b2s[:])
        for r in range(1, min(3, NR)):
            dma_in(r)
        emit_mm1(0)
        # steady state: iteration q emits mm1(q+1), h1+mm2(q), h2(q-1)
        for q in range(NQ):
            r, s, co, w = chunks[q]
            # prefetch region r+3 once every mm1 read of region r is emitted
            # (its xin buffer slot is the one being recycled)
            if s == B - 1 and r + 3 < NR:
                dma_in(r + 3)
            if q + 1 < NQ:
                emit_mm1(q + 1)
            emit_h1_mm2(q)
            if q > 0:
                emit_h2(q - 1)
        emit_h2(NQ - 1)

    nc.finalize()
    return nc


# ----------------------------------------------------------------------------
# Entry point
# ----------------------------------------------------------------------------

def _maybe_install_ntff_hook():
    try:
        import antenv.axon_hooks  # noqa: F401
        return
    except ImportError:
        pass
    try:
        from trn_agent_boot.trn_boot import _ntff_profile_via_ctypes
        hook = _ntff_profile_via_ctypes("/opt/axon/libaxon_pjrt.so")
        mod = types.ModuleType("antenv.axon_hooks")
        mod.get_axon_ntff_profile_hook = lambda: hook
        mod.set_axon_ntff_profile_hook = lambda h: None
        sys.modules["antenv.axon_hooks"] = mod
    except Exception:
        pass


def kernel(x, h_node, W1, b1, W2, b2, W3, b3, ptr):
    global LAST_RESULT
    x = np.asarray(x, np.float32)
    h_node = np.asarray(h_node, np.float32)
    W1 = np.asarray(W1, np.float32)
    W2 = np.asarray(W2, np.float32)
    W3 = np.asarray(W3, np.float32)
    b1 = np.asarray(b1, np.float32)
    b2 = np.asarray(b2, np.float32)
    b3 = np.asarray(b3, np.float32)
    ptr = np.asarray(ptr).astype(np.int64)
    N, F = x.shape
    E = h_node.shape[1]
    M = ptr.shape[0] - 1

    cnts = np.diff(ptr)

    # per-core/stream block arrays and the common padded block count
    bounds = _stream_bounds(cnts)
    core_streams = []
    blk_max = 0
    for c in range(NCORES):
        streams = []
        for st in range(2):
            l2, h2 = bounds[2 * c + st], bounds[2 * c + st + 1]
            blk_src, blk_cnt, nb_per_seg = _stream_blocks(
                ptr[l2:h2], cnts[l2:h2])
            streams.append((l2, h2, blk_src, blk_cnt, nb_per_seg))
            blk_max = max(blk_max, len(blk_src))
        core_streams.append(streams)

    blocks_padded = -(-blk_max // W_GRAN) * W_GRAN
    plan = _region_plan(blocks_padded)
    TOTB = blocks_padded
    C = TOTB * B

    # device weight/constant tensors
    w1blk = np.zeros((80, 128), np.float32)
    w1blk[0:40, 0:64] = W1
    w1blk[40:80, 64:128] = W1
    w2blk = np.zeros((128, 128), np.float32)
    w2blk[0:64, 0:64] = W2
    w2blk[64:128, 64:128] = W2
    b1st = np.concatenate([b1, b1]).reshape(128, 1).astype(np.float32)
    b2st = np.concatenate([b2, b2]).reshape(128, 1).astype(np.float32)

    in_maps = []
    for c in range(NCORES):
        xcat = np.zeros((80, C), BF16)
        for st, (l2, h2, blk_src, blk_cnt, nb_per_seg) in \
                enumerate(core_streams[c]):
            bs = np.full(TOTB, -1, np.int64)
            bc = np.zeros(TOTB, np.int64)
            bs[:len(blk_src)] = blk_src
            bc[:len(blk_cnt)] = blk_cnt
            src = _col_src(bs, bc, plan)
            r0 = 40 * st
            xcat[r0:r0 + 8, :] = _gather_T(x, src)
            xcat[r0 + 8:r0 + 40, :] = _gather_T(h_node, src)
        in_maps.append({
            "xcat": xcat,
            "w1blk": w1blk.astype(BF16),
            "w2blk": w2blk.astype(BF16),
            "b1s": b1st,
            "b2s": b2st,
        })

    zero_b2 = bool(np.all(b2 == 0.0))
    nc = _build_device_program(plan, zero_b2)
    _maybe_install_ntff_hook()
    res = run_bass_kernel_spmd(nc, in_maps, core_ids=list(range(NCORES)))
    LAST_RESULT = res

    # host assembly: block partials -> segment sums -> @W3 + bias terms
    out = np.zeros((M, E), np.float32)
    # each empty slot inside a real block contributed relu(relu(b1)@W2 + b2)
    h2c = np.maximum(np.maximum(b1, 0.0) @ W2 + b2, 0.0)
    corr = (h2c @ W3).astype(np.float32)  # [E]
    last_off, w_last = plan[-1]
    for c in range(NCORES):
        P = np.asarray(res.results[c]["outT"], np.float32)  # [128, TOTB]
        P[:, last_off:last_off + w_last] = (
            np.asarray(res.results[c]["outA"], np.float32)
            + np.asarray(res.results[c]["outB"], np.float32))
        for st, (l2, h2, blk_src, blk_cnt, nb_per_seg) in \
                enumerate(core_streams[c]):
            nb = len(blk_src)
            p_st = P[st * 64:(st + 1) * 64, :nb].T  # [nb, 64]
            cs = np.concatenate([np.zeros((1, 64), np.float64),
                                 np.cumsum(p_st, axis=0, dtype=np.float64)])
            ends = np.cumsum(nb_per_seg)
            starts = ends - nb_per_seg
            h2sum = (cs[ends] - cs[starts]).astype(np.float32)  # [nsegs, 64]
            segs = np.arange(l2, h2)
            pad_slots = (B * nb_per_seg - cnts[l2:h2]).astype(np.float32)
            out[segs] = (h2sum @ W3
                         + cnts[l2:h2, None].astype(np.float32) * b3[None, :]
                         - pad_slots[:, None] * corr[None, :])
    return out
